# revision 4
# baseline (speedup 1.0000x reference)
"""SlotAttention kernel for 8 Trainium2 NeuronCores.

Sharding: data-parallel over batch (B=32 -> 4 per core), all params
(<100KB) replicated on every core, per the problem's sharding hint.
Each core runs the full 3-iteration slot-attention loop on its batch
shard; outputs are gathered on host. No cross-core communication is
needed because every tensor contraction is within a single batch
element.
"""
import numpy as np
import jax
import jax.numpy as jnp

B, N, K = 32, 16384, 11
D_IN, D_C, D_S, HID = 64, 64, 64, 128
N_ITER = 3
EPS_LN = 1e-5
NCORES = 8

_PARAM_NAMES = ('mu', 'sigma', 'Wq', 'bq', 'Wk', 'bk', 'Wv', 'bv',
                'W_ih', 'b_ih', 'W_hh', 'b_hh', 'W1', 'b1', 'W2', 'b2')


def _ln(x):
    m = jnp.mean(x, axis=-1, keepdims=True)
    v = jnp.mean((x - m) ** 2, axis=-1, keepdims=True)
    return (x - m) / jnp.sqrt(v + EPS_LN)


def _shard_fn(inputs, slot_noise, mu, sigma, Wq, bq, Wk, bk, Wv, bv,
              W_ih, b_ih, W_hh, b_hh, W1, b1, W2, b2):
    # inputs: [b, N, D_IN], slot_noise: [b, K, D_C] for this core's shard
    b = inputs.shape[0]
    slots = mu + sigma * slot_noise                      # [b,K,Ds]
    x = _ln(inputs)                                      # [b,N,D]
    sx = jnp.sum(x, axis=1)                              # [b,D] (iter-invariant)
    scale = 1.0 / np.sqrt(D_C)
    attn_t = None
    for it in range(N_ITER):
        q = _ln(slots) @ Wq.T + bq                       # [b,K,Dc]
        # scores.T[k,n] = scale*(k_proj[n]@q[k]) with k_proj = x@Wk.T + bk
        #              = scale*(x[n]@(Wk.T q[k]) + bk@q[k])
        qeff = (q @ Wk) * scale                          # [b,K,D]
        c0 = scale * (q @ bk)                            # [b,K]
        scores_t = jnp.einsum('bkd,bnd->bkn', qeff, x) + c0[:, :, None]
        # softmax over slots without max-subtraction: |scores| <~ 10 since x is
        # LayerNormed and qeff is small, so exp cannot overflow in fp32.
        w = jnp.exp(scores_t)                            # [b,K,N]
        p = w / jnp.sum(w, axis=1, keepdims=True)        # softmax over k
        # attn = p + 1e-8, then renorm over n:
        colsum = jnp.sum(p, axis=2, keepdims=True) + 1e-8 * N   # [b,K,1]
        # updates[k] = sum_n (p+1e-8)[k,n]/colsum[k] * (x[n]@Wv.T + bv)
        #            = ((p@x + 1e-8*sx) / colsum) @ Wv.T + bv
        u0 = (jnp.einsum('bkn,bnd->bkd', p, x) + 1e-8 * sx[:, None, :]) / colsum
        updates = u0 @ Wv.T + bv
        if it == N_ITER - 1:
            attn_t = (p + 1e-8) / colsum                 # full masks, last iter only
        xg = updates.reshape(-1, D_C) @ W_ih.T + b_ih
        hg = slots.reshape(-1, D_S) @ W_hh.T + b_hh
        xr, xz, xn = jnp.split(xg, 3, axis=-1)
        hr, hz, hn = jnp.split(hg, 3, axis=-1)
        r = jax.nn.sigmoid(xr + hr)
        z = jax.nn.sigmoid(xz + hz)
        n = jnp.tanh(xn + r * hn)
        h = (1.0 - z) * n + z * slots.reshape(-1, D_S)
        slots = _ln(h.reshape(b, K, D_S))
        slots = slots + (jax.nn.relu(slots @ W1.T + b1) @ W2.T + b2)
    masks = attn_t.reshape(b, K, 128, 128)  # attn_t is already [b,K,N]
    return slots, masks


_pmapped = None


def _get_pmapped():
    global _pmapped
    if _pmapped is None:
        _pmapped = jax.pmap(
            _shard_fn,
            in_axes=(0, 0) + (None,) * len(_PARAM_NAMES),
            devices=jax.devices()[:NCORES],
        )
    return _pmapped


def kernel(**inputs):
    x = np.ascontiguousarray(inputs['inputs'], dtype=np.float32)
    sn = np.ascontiguousarray(inputs['slot_noise'], dtype=np.float32)
    per = B // NCORES
    x_sh = x.reshape(NCORES, per, N, D_IN)
    sn_sh = sn.reshape(NCORES, per, K, D_C)
    params = [np.asarray(inputs[p], dtype=np.float32) for p in _PARAM_NAMES]
    slots, masks = _get_pmapped()(x_sh, sn_sh, *params)
    slots = np.asarray(slots, dtype=np.float32).reshape(B, K, D_S)
    masks = np.asarray(masks, dtype=np.float32).reshape(B, K, 128, 128)
    return slots, masks


# revision 7
# speedup vs baseline: 1.0047x; 1.0047x over previous
"""SlotAttention kernel for 8 Trainium2 NeuronCores.

Sharding: data-parallel over batch (B=32 -> 4 per core), all params
(<100KB) replicated on every core, per the problem's sharding hint.
Each core runs the full 3-iteration slot-attention loop on its batch
shard; outputs are gathered on host. No cross-core communication is
needed because every tensor contraction is within a single batch
element.
"""
import numpy as np
import jax
import jax.numpy as jnp

B, N, K = 32, 16384, 11
D_IN, D_C, D_S, HID = 64, 64, 64, 128
N_ITER = 3
EPS_LN = 1e-5
NCORES = 8

_PARAM_NAMES = ('mu', 'sigma', 'Wq', 'bq', 'Wk', 'bk', 'Wv', 'bv',
                'W_ih', 'b_ih', 'W_hh', 'b_hh', 'W1', 'b1', 'W2', 'b2')


def _ln(x):
    m = jnp.mean(x, axis=-1, keepdims=True)
    v = jnp.mean((x - m) ** 2, axis=-1, keepdims=True)
    return (x - m) / jnp.sqrt(v + EPS_LN)


def _shard_fn(inputs, slot_noise, mu, sigma, Wq, bq, Wk, bk, Wv, bv,
              W_ih, b_ih, W_hh, b_hh, W1, b1, W2, b2):
    # inputs: [b, N, D_IN], slot_noise: [b, K, D_C] for this core's shard
    b = inputs.shape[0]
    slots = mu + sigma * slot_noise                      # [b,K,Ds]
    x = _ln(inputs)                                      # [b,N,D]
    sx = jnp.sum(x, axis=1)                              # [b,D] (iter-invariant)
    # bf16 copy of x for the two big einsums (fp32 accumulation): halves the
    # dominant HBM traffic in this memory-bound kernel; attention stays fp32.
    x16 = x.astype(jnp.bfloat16)
    scale = 1.0 / np.sqrt(D_C)
    attn_t = None
    for it in range(N_ITER):
        q = _ln(slots) @ Wq.T + bq                       # [b,K,Dc]
        # scores.T[k,n] = scale*(k_proj[n]@q[k]) with k_proj = x@Wk.T + bk
        #              = scale*(x[n]@(Wk.T q[k]) + bk@q[k])
        qeff = (q @ Wk) * scale                          # [b,K,D]
        c0 = scale * (q @ bk)                            # [b,K]
        scores_t = jnp.einsum('bkd,bnd->bkn', qeff.astype(jnp.bfloat16), x16,
                              preferred_element_type=jnp.float32) + c0[:, :, None]
        # softmax over slots without max-subtraction: |scores| <~ 10 since x is
        # LayerNormed and qeff is small, so exp cannot overflow in fp32.
        w = jnp.exp(scores_t)                            # [b,K,N]
        p = w / jnp.sum(w, axis=1, keepdims=True)        # softmax over k
        # attn = p + 1e-8, then renorm over n:
        colsum = jnp.sum(p, axis=2, keepdims=True) + 1e-8 * N   # [b,K,1]
        # updates[k] = sum_n (p+1e-8)[k,n]/colsum[k] * (x[n]@Wv.T + bv)
        #            = ((p@x + 1e-8*sx) / colsum) @ Wv.T + bv
        u0 = (jnp.einsum('bkn,bnd->bkd', p.astype(jnp.bfloat16), x16,
                         preferred_element_type=jnp.float32)
              + 1e-8 * sx[:, None, :]) / colsum
        updates = u0 @ Wv.T + bv
        if it == N_ITER - 1:
            attn_t = (p + 1e-8) / colsum                 # full masks, last iter only
        xg = updates.reshape(-1, D_C) @ W_ih.T + b_ih
        hg = slots.reshape(-1, D_S) @ W_hh.T + b_hh
        xr, xz, xn = jnp.split(xg, 3, axis=-1)
        hr, hz, hn = jnp.split(hg, 3, axis=-1)
        r = jax.nn.sigmoid(xr + hr)
        z = jax.nn.sigmoid(xz + hz)
        n = jnp.tanh(xn + r * hn)
        h = (1.0 - z) * n + z * slots.reshape(-1, D_S)
        slots = _ln(h.reshape(b, K, D_S))
        slots = slots + (jax.nn.relu(slots @ W1.T + b1) @ W2.T + b2)
    masks = attn_t.reshape(b, K, 128, 128)  # attn_t is already [b,K,N]
    return slots, masks


_pmapped = None


def _get_pmapped():
    global _pmapped
    if _pmapped is None:
        _pmapped = jax.pmap(
            _shard_fn,
            in_axes=(0, 0) + (None,) * len(_PARAM_NAMES),
            devices=jax.devices()[:NCORES],
        )
    return _pmapped


def kernel(**inputs):
    x = np.ascontiguousarray(inputs['inputs'], dtype=np.float32)
    sn = np.ascontiguousarray(inputs['slot_noise'], dtype=np.float32)
    per = B // NCORES
    x_sh = x.reshape(NCORES, per, N, D_IN)
    sn_sh = sn.reshape(NCORES, per, K, D_C)
    params = [np.asarray(inputs[p], dtype=np.float32) for p in _PARAM_NAMES]
    slots, masks = _get_pmapped()(x_sh, sn_sh, *params)
    slots = np.asarray(slots, dtype=np.float32).reshape(B, K, D_S)
    masks = np.asarray(masks, dtype=np.float32).reshape(B, K, 128, 128)
    return slots, masks
